# revision 31
# baseline (speedup 1.0000x reference)
"""Trainium2 Bass kernel for MLP-with-SOM-cosine-similarity (retrieval_knn).

Reference computation per (b, k) pair:
  ctx, ent: [L=128, D=128] slices of context[b, k, 0/1]
  sim[l, m] = cos(ctx[l], ent[m]); idx[l] = argmax_m sim[l, m]
  x = [ctx_n | ent_n[idx]] -> 6x tanh(Linear(256,256)) -> dot W_out -> sum over l
Output: [B=64, K=64] f32.

Strategy: data-parallel over batch dim (8 cores x 8 batches = 512 pairs/core).

The span is bound by the ACT (Scalar) tanh conveyor: 24 x [128,1024]
activations per 16-pair subgroup = ~26.7us; everything else must hide behind
it. Design rules learned from traces:
  - ACT queue holds ONLY the 768 tanhs.
  - The in-order PE queue must never hold an instruction whose inputs are
    not ready: every producer->consumer crossing (PE->DVE->PE) gets >= ~3
    mm-tile slots (~3.3us) of distance.
  - Filler work is emitted in <= ~1us chunks after each of the 24 mm-tiles,
    so the next mm never sits behind a long filler (ACT starves ~0.9us per
    occurrence otherwise).
  - Long DVE batch ops (norm reduce, Newton, y6 row-sums) are spread over
    the tail slots where sim-phase DVE traffic is light, never at the head
    of a subgroup (they used to stall the first MLP layer and downclock the
    PE: idle PE drops 2.4->1.2GHz and halves matmul throughput).
  - gpsimd TT (~1.9ns/el) beats gpsimd CAST (~3.5ns/el): bf16 copies of the
    normalized tensors are second normalize-TTs with bf16 output.
  - ctx_nT bf16 (MLP chunk0) comes from a DMA xbar transpose of ctxb
    (16-bit only), one [128,512] call per 4-pair group.
Pipeline (subgroup w): DMA+squares at w-3 | norms spread over w-2 | sim
stages A1 (fp32 transposes), A2 (sim+argmax), B1 (one-hot transpose), B2
(gather) at slots 3k/3k+4/3k+7/3k+10 of mlp(w-1) | mlp at w | y6 row-sums at
slots 23/next-0 | tiny wout matmuls at slot 22 of w+1.
PSUM banks: tp(2) + sim(1) + scr(1) + mlp(4, also hosts tiny wo) = 8.
"""

from collections import defaultdict
from contextlib import ExitStack

import numpy as np
import ml_dtypes

import concourse.bass as bass
import concourse.bacc as bacc
import concourse.tile as tile
from concourse import mybir
from concourse.alu_op_type import AluOpType
from concourse.bass_utils import run_bass_kernel_spmd
from concourse.masks import make_identity

BF16 = mybir.dt.bfloat16
F32 = mybir.dt.float32
AF = mybir.ActivationFunctionType

B, K, L, D = 64, 64, 128, 128
N_CORES = 8
PAIRS = (B // N_CORES) * K          # 512 pairs per core
N_HIDDEN = 6
SUB = 16                            # pairs per DMA subgroup
GRP = 4                             # pairs per PSUM group
UNROLL = 128                        # pairs per outer block

_cache = {}


def _build_bass():
    nc = bacc.Bacc("TRN2")

    ctx_dram = nc.dram_tensor("ctxpairs", [PAIRS, 2, L, D], F32, kind="ExternalInput")
    wt_dram = nc.dram_tensor("wt", [128, N_HIDDEN * 2 * 2 * 128], BF16, kind="ExternalInput")
    wout_dram = nc.dram_tensor("wout", [128, 2], BF16, kind="ExternalInput")
    bias_dram = nc.dram_tensor("bias", [128, N_HIDDEN * 2], F32, kind="ExternalInput")
    bout_dram = nc.dram_tensor("bout", [1, 1], F32, kind="ExternalInput")
    out_dram = nc.dram_tensor("out", [1, PAIRS], F32, kind="ExternalOutput")

    with ExitStack() as ctx:
        tc = ctx.enter_context(tile.TileContext(nc))
        const = ctx.enter_context(tc.tile_pool(name="const", bufs=1))
        raw_pool = ctx.enter_context(tc.tile_pool(name="raw", bufs=4))
        sq_pool = ctx.enter_context(tc.tile_pool(name="sq", bufs=2))
        norm_pool = ctx.enter_context(tc.tile_pool(name="norm", bufs=2))
        tiny_pool = ctx.enter_context(tc.tile_pool(name="tiny", bufs=4))
        pre_pool = ctx.enter_context(tc.tile_pool(name="pre", bufs=4))
        x_pool = ctx.enter_context(tc.tile_pool(name="xsb", bufs=6))
        y_pool = ctx.enter_context(tc.tile_pool(name="ysb", bufs=4))
        ybar_pool = ctx.enter_context(tc.tile_pool(name="ybar", bufs=4))
        res_pool = ctx.enter_context(tc.tile_pool(name="res", bufs=2))
        ps_tp = ctx.enter_context(tc.tile_pool(name="pstp", bufs=1, space="PSUM"))
        ps_sim = ctx.enter_context(tc.tile_pool(name="pssim", bufs=1, space="PSUM"))
        ps_scr = ctx.enter_context(tc.tile_pool(name="psscr", bufs=1, space="PSUM"))
        ps_mlp = ctx.enter_context(tc.tile_pool(name="psmlp", bufs=2, space="PSUM"))

        wt_sb = const.tile([128, N_HIDDEN, 2, 2, 128], BF16)
        nc.sync.dma_start(out=wt_sb, in_=wt_dram.rearrange("a (i kc mc b) -> a i kc mc b", i=N_HIDDEN, kc=2, mc=2))
        wout_sb = const.tile([128, 2], BF16)
        nc.sync.dma_start(out=wout_sb, in_=wout_dram[:, :])
        bias_sb = const.tile([128, N_HIDDEN * 2], F32)
        nc.sync.dma_start(out=bias_sb, in_=bias_dram[:, :])
        bout_sb = const.tile([1, 1], F32)
        nc.sync.dma_start(out=bout_sb, in_=bout_dram[:, :])
        ident = const.tile([128, 128], F32)
        make_identity(nc, ident)
        identb = const.tile([128, 128], BF16)
        make_identity(nc, identb)
        bout128 = const.tile([1, 1], F32)
        nc.vector.tensor_scalar(out=bout128, in0=bout_sb, scalar1=float(L), scalar2=0.0,
                                op0=AluOpType.mult, op1=AluOpType.add)

        n_blk = UNROLL // SUB
        n_sub_total = PAIRS // SUB
        HS = SUB // 2

        def dma_stage(s):
            raw = raw_pool.tile([128, SUB, 2, 128], F32, tag="raw")
            nc.sync.dma_start(
                out=raw,
                in_=ctx_dram[s * SUB : s * SUB + SUB].rearrange("p c l d -> l p c d"),
            )
            return raw

        def sq_stage(raw):
            sq = sq_pool.tile([128, SUB, 2, 128], F32, tag="sq")
            for hh in range(2):
                sl = slice(hh * HS, hh * HS + HS)
                nc.gpsimd.tensor_mul(sq[:, sl], raw[:, sl], raw[:, sl])
            return sq

        # --- norm chain, split into slot-sized pieces ---
        def norm_reduce_h(rawsq, nrm2, hh):
            raw, sq = rawsq
            sl = slice(hh * HS, hh * HS + HS)
            nc.vector.tensor_reduce(nrm2[:, sl], sq[:, sl], axis=mybir.AxisListType.X, op=AluOpType.add)

        def norm_newton(nrm2):
            nrm2f = nrm2.rearrange("a p c -> a (p c)")
            nc.vector.tensor_scalar(out=nrm2f, in0=nrm2f, scalar1=1.0 / 128.0,
                                    scalar2=0.0, op0=AluOpType.mult, op1=AluOpType.add)
            yv = tiny_pool.tile([128, SUB, 2], F32, tag="newty")
            tv = tiny_pool.tile([128, SUB, 2], F32, tag="newtt")
            yvf = yv.rearrange("a p c -> a (p c)")
            tvf = tv.rearrange("a p c -> a (p c)")
            nc.vector.tensor_scalar(out=yvf, in0=nrm2f, scalar1=-0.5, scalar2=1.5,
                                    op0=AluOpType.mult, op1=AluOpType.add)
            for _ in range(2):
                nc.vector.tensor_mul(tvf, yvf, yvf)
                nc.vector.tensor_mul(tvf, tvf, nrm2f)
                nc.vector.tensor_scalar(out=tvf, in0=tvf, scalar1=-0.5, scalar2=1.5,
                                        op0=AluOpType.mult, op1=AluOpType.add)
                nc.vector.tensor_mul(yvf, yvf, tvf)
            nc.vector.tensor_scalar(out=yvf, in0=yvf, scalar1=float(1.0 / np.sqrt(128.0)),
                                    scalar2=0.0, op0=AluOpType.mult, op1=AluOpType.add)
            return yv

        def norm_normalize(raw, yv):
            ctxn = norm_pool.tile([128, SUB, 128], F32, tag="ctxn")
            entn = norm_pool.tile([128, SUB, 128], F32, tag="entn")
            ctxb = norm_pool.tile([128, SUB, 128], BF16, tag="ctxb")
            entb = norm_pool.tile([128, SUB, 128], BF16, tag="entb")
            for hh in range(2):
                sl = slice(hh * HS, hh * HS + HS)
                rinv_c = yv[:, sl, 0:1].broadcast_to([128, HS, 128])
                rinv_e = yv[:, sl, 1:2].broadcast_to([128, HS, 128])
                nc.gpsimd.tensor_tensor(out=ctxn[:, sl], in0=raw[:, sl, 0, :], in1=rinv_c, op=AluOpType.mult)
                nc.gpsimd.tensor_tensor(out=entn[:, sl], in0=raw[:, sl, 1, :], in1=rinv_e, op=AluOpType.mult)
                nc.gpsimd.tensor_tensor(out=ctxb[:, sl], in0=raw[:, sl, 0, :], in1=rinv_c, op=AluOpType.mult)
                nc.gpsimd.tensor_tensor(out=entb[:, sl], in0=raw[:, sl, 1, :], in1=rinv_e, op=AluOpType.mult)
            return ctxn, entn, ctxb, entb

        # --- sim pipeline, 4 sub-stages of <= ~1us PE work each ---
        def stage_a1(st, q):
            """fp32 transposes of 4 pairs (ctx+ent) + one big PSUM->SBUF copy
            + ctx chunk0 via DMA xbar transpose of ctxb."""
            ctxn, entn, ctxb, entb = st
            pbase = q * GRP
            tp = ps_tp.tile([128, 2, GRP, 128], F32, tag="tp")
            for j in range(GRP):
                p = pbase + j
                nc.tensor.transpose(tp[:, 0, j, :], ctxn[:, p, :], ident)
                nc.tensor.transpose(tp[:, 1, j, :], entn[:, p, :], ident)
            cpt = pre_pool.tile([128, 2, GRP, 128], F32, tag="cpt")
            nc.vector.tensor_copy(cpt, tp)
            x_sb = x_pool.tile([128, 2, GRP, 128], BF16, tag="x")
            nc.sync.dma_start_transpose(
                x_sb[:, 0, :, :],
                ctxb[:, pbase : pbase + GRP, :].rearrange("a p d -> a (p d)"),
            )
            return {"cpt": cpt, "x": x_sb, "pbase": pbase}

        def stage_a2(ab):
            """sim matmuls (fp32) + argmax one-hot on DVE."""
            cpt = ab["cpt"]
            sim = ps_sim.tile([128, GRP, 128], F32, tag="sim")
            for j in range(GRP):
                nc.tensor.matmul(sim[:, j, :], lhsT=cpt[:, 0, j, :], rhs=cpt[:, 1, j, :])
            mx = tiny_pool.tile([128, GRP], F32, tag="mx")
            nc.vector.tensor_reduce(mx, sim, axis=mybir.AxisListType.X, op=AluOpType.max)
            oh = pre_pool.tile([128, GRP, 128], BF16, tag="oh")
            nc.vector.tensor_tensor(
                out=oh, in0=sim,
                in1=mx.unsqueeze(2).broadcast_to([128, GRP, 128]),
                op=AluOpType.is_equal,
            )
            ab["oh"] = oh

        def stage_b1(ab):
            """one-hot transpose via DMA xbar (idle engine; oh has had ~3us
            to land, so the in-order SP queue never holds a waiting instr)."""
            oh = ab["oh"]
            ohT = pre_pool.tile([128, GRP, 128], BF16, tag="ohT")
            nc.sync.dma_start_transpose(ohT, oh.rearrange("a p d -> a (p d)"))
            ab["ohT"] = ohT

        def stage_b2(st, ab):
            """gather + gathered chunk cast."""
            ctxn, entn, ctxb, entb = st
            x_sb, ohT, pbase = ab["x"], ab["ohT"], ab["pbase"]
            gat = ps_scr.tile([128, GRP, 128], F32, tag="scr")
            for j in range(GRP):
                nc.tensor.matmul(gat[:, j, :], lhsT=entb[:, pbase + j, :], rhs=ohT[:, j, :])
            nc.vector.tensor_copy(x_sb[:, 1], gat)
            return x_sb

        def emit_ybar_q(ya_last, qq):
            """Sum y6 over l per (pair, mc) for one supergroup + bf16 copy."""
            ybar = ybar_pool.tile([128, 2, 2, GRP], F32, tag="ybar")
            ybarb = ybar_pool.tile([128, 2, 2, GRP], BF16, tag="ybarb")
            ya_v = ya_last[qq].rearrange("a mc g (p l) -> a mc g p l", p=GRP)
            for mc in range(2):
                nc.vector.tensor_reduce(ybar[:, mc], ya_v[:, mc],
                                        axis=mybir.AxisListType.X, op=AluOpType.add)
            nc.vector.tensor_copy(ybarb, ybar)
            return ybarb

        def emit_wout_for(s, ybarbs, res):
            for qq in range(2):
                wo = ps_mlp.tile([1, 2, GRP], F32, tag="mm")
                for mc in range(2):
                    nc.tensor.matmul(wo.rearrange("a t g -> a (t g)"),
                                     lhsT=wout_sb[:, mc : mc + 1],
                                     rhs=ybarbs[qq][:, mc].rearrange("a g p -> a (g p)"),
                                     start=(mc == 0), stop=(mc == 1))
                col = (s % n_blk) * SUB + qq * 2 * GRP
                nc.vector.tensor_copy(res[0:1, col : col + 2 * GRP],
                                      wo.rearrange("a t g -> a (t g)"))

        def finalize_res(res, blk):
            g0 = blk * UNROLL
            nc.vector.tensor_scalar(out=res, in0=res, scalar1=bout128[0:1, 0:1],
                                    scalar2=0.0, op0=AluOpType.add, op1=AluOpType.add)
            nc.sync.dma_start(out=out_dram[0:1, g0 : g0 + UNROLL], in_=res)

        def mlp_subgroup(s, x_tiles, slots, ya_lasts):
            """MLP for 16 pairs; slots[j] callbacks run after mm-tile j
            (j=0..23) so fillers interleave at ~0.9us granularity."""
            xins = [
                [[x_tiles[2 * qq + g][:, kc].rearrange("a g d -> a (g d)") for kc in range(2)]
                 for g in range(2)]
                for qq in range(2)
            ]
            idx = 0
            for i in range(N_HIDDEN):
                yas = []
                for qq in range(2):
                    ya = y_pool.tile([128, 2, 2, GRP * 128], BF16, tag="y")
                    yas.append(ya)
                if i == N_HIDDEN - 1:
                    ya_lasts[s] = yas   # visible to the slot-23 callback
                for mc in range(2):
                    for qq in range(2):
                        mm = ps_mlp.tile([128, 2, GRP * 128], F32, tag="mm")
                        for g in range(2):
                            nc.tensor.matmul(mm[:, g, :], lhsT=wt_sb[:, i, 0, mc, :],
                                             rhs=xins[qq][g][0], start=True, stop=False)
                            nc.tensor.matmul(mm[:, g, :], lhsT=wt_sb[:, i, 1, mc, :],
                                             rhs=xins[qq][g][1], start=False, stop=True)
                        nc.scalar.activation(
                            out=yas[qq][:, mc].rearrange("a g d -> a (g d)"),
                            in_=mm.rearrange("a g d -> a (g d)"),
                            func=AF.Tanh,
                            bias=bias_sb[:, 2 * i + mc : 2 * i + mc + 1],
                        )
                        for cb in slots.get(idx, ()):
                            cb()
                        idx += 1
                xins = [[[yas[qq][:, kc, g] for kc in range(2)] for g in range(2)]
                        for qq in range(2)]
            return yas

        # ---------- software pipeline ----------
        raws = {w: dma_stage(w) for w in (0, 1, 2)}
        sqs = {w: sq_stage(raws[w]) for w in (0, 1, 2)}
        sts = {}
        for w in (0, 1):
            nrm2 = tiny_pool.tile([128, SUB, 2], F32, tag="nrm2")
            for hh in range(2):
                norm_reduce_h((raws[w], sqs[w]), nrm2, hh)
            yv = norm_newton(nrm2)
            sts[w] = norm_normalize(raws[w], yv)

        x_cur = []
        for q in range(SUB // GRP):
            ab = stage_a1(sts[0], q)
            stage_a2(ab)
            stage_b1(ab)
            x_cur.append(stage_b2(sts[0], ab))

        res = None
        ybarbs = {}          # s -> [q0, q1]
        ya_lasts = {}
        res_of = {}
        for s in range(n_sub_total):
            if s % n_blk == 0:
                res = res_pool.tile([1, UNROLL], F32, tag="res")
            res_of[s] = res
            slots = defaultdict(list)

            # previous subgroup's second y6 row-sum at slot 0 (after cpt copy)
            if s - 1 >= 0:
                def eb_q1(_s=s - 1):
                    ybarbs[_s].append(emit_ybar_q(ya_lasts[_s], 1))
                slots[0].append(eb_q1)

            # sim stages for subgroup s+1
            if s + 1 < n_sub_total:
                st_next = sts[s + 1]
                abs_ = {}
                for k in range(4):
                    def a1(_k=k, _st=st_next):
                        abs_[_k] = stage_a1(_st, _k)

                    def a2(_k=k):
                        stage_a2(abs_[_k])

                    def b1(_k=k):
                        stage_b1(abs_[_k])

                    def b2(_k=k, _st=st_next, _x=None):
                        x_next.append(stage_b2(_st, abs_[_k]))

                    slots[3 * k + 0].append(a1)
                    slots[3 * k + 4].append(a2)
                    slots[3 * k + 7].append(b1)
                    slots[3 * k + 10].append(b2)
            x_next = []

            # norm chain for subgroup s+2, spread over tail slots
            if s + 2 < n_sub_total:
                nrm2_t = tiny_pool.tile([128, SUB, 2], F32, tag="nrm2")

                def nr0(_w=s + 2, _t=nrm2_t):
                    norm_reduce_h((raws[_w], sqs[_w]), _t, 0)

                def nr1(_w=s + 2, _t=nrm2_t):
                    norm_reduce_h((raws[_w], sqs[_w]), _t, 1)

                def nwt(_w=s + 2, _t=nrm2_t):
                    yv = norm_newton(_t)
                    sts[_w] = norm_normalize(raws[_w], yv)

                slots[14].append(nr0)
                slots[16].append(nr1)
                slots[18].append(nwt)

            # DMA + squares for subgroup s+3
            if s + 3 < n_sub_total:
                def dsq(_w=s + 3):
                    raws[_w] = dma_stage(_w)
                    sqs[_w] = sq_stage(raws[_w])
                slots[20].append(dsq)

            # previous subgroup's wout + res finalize at slot 22
            if s - 1 >= 0:
                def wo_cb(_s=s - 1):
                    emit_wout_for(_s, ybarbs[_s], res_of[_s])
                    if _s % n_blk == n_blk - 1:
                        finalize_res(res_of[_s], _s // n_blk)
                slots[22].append(wo_cb)

            # this subgroup's first y6 row-sum at slot 23
            def eb_q0(_s=s):
                ybarbs[_s] = [emit_ybar_q(ya_lasts[_s], 0)]
            slots[23].append(eb_q0)

            mlp_subgroup(s, x_cur, slots, ya_lasts)
            x_cur = x_next

        # epilogue
        s_last = n_sub_total - 1
        ybarbs[s_last].append(emit_ybar_q(ya_lasts[s_last], 1))
        emit_wout_for(s_last, ybarbs[s_last], res_of[s_last])
        finalize_res(res_of[s_last], s_last // n_blk)

    nc.compile()
    return nc


def _prep_weights(Ws, bs, W_out, b_out):
    Ws = np.asarray(Ws, dtype=np.float32)
    bs = np.asarray(bs, dtype=np.float32)
    W_out = np.asarray(W_out, dtype=np.float32)
    b_out = np.asarray(b_out, dtype=np.float32)
    # wt[a, i, kc, mc, b] = Ws[i, mc*128+b, kc*128+a]
    wt = np.transpose(
        Ws.reshape(N_HIDDEN, 2, 128, 2, 128),  # [i, mc, b, kc, a]
        (4, 0, 3, 1, 2),
    ).reshape(128, N_HIDDEN * 2 * 2 * 128)
    wt = np.ascontiguousarray(wt.astype(ml_dtypes.bfloat16))
    wout = np.ascontiguousarray(W_out.reshape(2, 128).T.astype(ml_dtypes.bfloat16))
    bias = np.ascontiguousarray(
        np.transpose(bs.reshape(N_HIDDEN, 2, 128), (2, 0, 1)).reshape(128, N_HIDDEN * 2)
    ).astype(np.float32)
    bout = b_out.reshape(1, 1).astype(np.float32)
    return wt, wout, bias, bout


def make_in_maps(context, Ws, bs, W_out, b_out):
    context = np.ascontiguousarray(np.asarray(context, dtype=np.float32))
    wt, wout, bias, bout = _prep_weights(Ws, bs, W_out, b_out)
    shards = context.reshape(N_CORES, PAIRS, 2, L, D)
    return [
        {"ctxpairs": np.ascontiguousarray(shards[i]), "wt": wt, "wout": wout,
         "bias": bias, "bout": bout}
        for i in range(N_CORES)
    ]


def kernel(context, Ws, bs, W_out, b_out):
    in_maps = make_in_maps(context, Ws, bs, W_out, b_out)
    if "nc" not in _cache:
        _cache["nc"] = _build_bass()
    nc = _cache["nc"]
    r = run_bass_kernel_spmd(nc, in_maps, core_ids=list(range(N_CORES)))
    out = np.concatenate([r.results[i]["out"].reshape(B // N_CORES, K) for i in range(N_CORES)], axis=0)
    return out.astype(np.float32)


if __name__ == "__main__":
    import reference
    inputs = reference.setup_inputs()
    inputs = {k: np.asarray(v) for k, v in inputs.items()}
    expected = np.asarray(reference.reference(**inputs))
    actual = kernel(**inputs)
    err = np.linalg.norm(actual - expected) / np.linalg.norm(expected)
    print("Relative error:", err)
